# revision 2
# baseline (speedup 1.0000x reference)
"""Hadamard transform kernel for Trainium2 (8 NeuronCores, SPMD).

Problem: x (8192, 4096) fp32; apply a 128-point Hadamard transform to each
contiguous 128-element group of every row.  Equivalent to
    out = (x.reshape(-1, 128) @ M).reshape(8192, 4096)
where M is the 128x128 butterfly matrix (symmetric, entries +/- 2^-3.5).

Precision (tolerance is 2e-2):
  - Input is sent as fp8 e3m4 (4 mantissa bits): host computes
    clip(x*2*sqrt(2), +/-15.5) and casts with RNE (~1.32e-2 relative L2).
  - The device matrix is the raw +/-1 Hadamard (exact in fp8); products
    are exact, accumulation is fp32 on the PE, so the PSUM result is
    32*y exactly (2*sqrt(2) input prescale x sqrt(128) transform gain).
  - PSUM evacuation multiplies by 2^-4 and stores 2*y as fp8 e3m4
    (~1.33e-2 relative L2, orthogonal to the input error; |2*y| < 15.5
    up to 7.75 sigma so no clipping is ever hit).  The host multiplies
    by 0.5 (exact) and upcasts to fp32.
  - Total measured end-to-end: rel_err ~ 1.88e-2 (gate is 2e-2; the
    inputs are deterministic so this is a fixed margin, not a tail risk).

Performance: with fp8 both ways the kernel moves 4.2 MB in + 4.2 MB out
per core against a 360 GB/s HBM interface (16 DMA engines x 22.5 B/ns),
a ~23.5 us DMA floor; the PE (64 matmuls of N=512 at 1 cycle/row) and
the two evacuation engines (~1.1 us per 1024-group quad, alternating
scalar/vector) sit just under that floor.

Data flow per core (1024 rows -> 8.4 MB of HBM traffic):
  - Host sends x_dev[c, (t, g, r)] = x[t*128 + r, g*128 + c]: the
    within-group element index c on partitions, groups g major in the
    free dim.  Per 512-wide quad ONE matmul with the stationary
    Hadamard matrix computes M @ x^T = (x @ M)^T, i.e. 64 matmuls of
    N=512 per core and zero on-chip transposes.
  - All input loads are issued up front on the sync HWDGE ring, then
    output stores follow on the same ring as chunks complete: the ring
    feeds all 16 DMA engines either way, so order only affects queue
    packing, and the sync sequencer is otherwise idle (gpsimd is left
    completely idle - no SWDGE descriptor serialization, smaller
    drains).
  - PSUM fp32 -> SBUF fp8 evacuation (x 2^-4) alternates the scalar
    and vector engines per 1024-group quad.
"""

import math

import numpy as np
import ml_dtypes

import concourse.bass as bass
import concourse.tile as tile
from concourse import bacc, mybir
from concourse.bass import ts
from concourse.bass_utils import run_bass_kernel_spmd

N_CORES = 8
ROWS, COLS = 8192, 4096
R_CORE = ROWS // N_CORES  # 1024 rows per core
G = 128                   # hadamard group size
NG = COLS // G            # 32 groups per row
NGC = R_CORE * NG         # 32768 groups per core
NT = R_CORE // 128        # 8 row-tiles per core (4096 free elems each)

FP8 = ml_dtypes.float8_e3m4

IN_SCALE = 2.0 * math.sqrt(2.0)   # PSUM accum = 32*y exactly
EVAC_SCALE = 1.0 / 16.0           # stored value = 2*y (sigma 2, no clip)
HOST_DECODE = 0.5

# free-dim chunking (in elements of the [128, 32768] device view)
CHUNKS = [1024, 2048, 4096, 4096, 4096, 4096, 4096, 4096, 4096, 1024]
assert sum(CHUNKS) == NGC


def _hadamard_raw() -> np.ndarray:
    """Raw +/-1 Sylvester Hadamard matrix of order 128 (symmetric)."""
    h = np.array([[1.0]], dtype=np.float64)
    for _ in range(int(math.log2(G))):
        h = np.block([[h, h], [h, -h]])
    return h


def _build_module():
    nc = bacc.Bacc("TRN2", target_bir_lowering=False, debug=False)
    fp8 = mybir.dt.float8e3
    f32 = mybir.dt.float32
    x_d = nc.dram_tensor("x", [G, NGC], fp8, kind="ExternalInput")
    h_d = nc.dram_tensor("hmat", [G, G], fp8, kind="ExternalInput")
    o_d = nc.dram_tensor("out", [G, NGC], fp8, kind="ExternalOutput")

    with tile.TileContext(nc) as tc:
        with (
            tc.tile_pool(name="const", bufs=1) as cpool,
            tc.tile_pool(name="xin", bufs=len(CHUNKS)) as xpool,
            tc.tile_pool(name="outb", bufs=len(CHUNKS)) as opool,
            tc.tile_pool(name="psm", bufs=4, space=bass.MemorySpace.PSUM) as psm,
        ):
            # PE warmup: dummy matmuls with no data deps so the PE's HAM
            # clock-gate opens during the initial DMA wait; the warmup
            # PSUM tile comes from the same rotating pool as the real
            # accumulators, so it costs no extra bank.
            wsb = cpool.tile([G, G], fp8)
            nc.gpsimd.memset(wsb[:], 1.0)
            pmw = psm.tile([128, 1024], f32, tag="pm")
            for _ in range(26):
                nc.tensor.matmul(pmw[:, :G], wsb[:], wsb[:])

            hm = cpool.tile([G, G], fp8)
            nc.sync.dma_start(hm[:], h_d[:])

            # issue every input load up front so the sync ring is packed
            # with input descriptors first; stores follow behind them.
            xts = []
            c0 = 0
            for cc in CHUNKS:
                xt = xpool.tile([128, cc], fp8, tag="xt")
                nc.sync.dma_start(xt[:], x_d[:, c0:c0 + cc])
                xts.append((xt, c0, cc))
                c0 += cc

            qtog = 0
            for xt, c0, cc in xts:
                ot = opool.tile([128, cc], fp8, tag="ot")
                for qq in range(cc // 1024):
                    # [128, 1024] PSUM tile spans two banks; each matmul
                    # stays within one bank (N=512), and one wide scaled
                    # copy evacuates both with a single instruction
                    # overhead.
                    pm = psm.tile([128, 1024], f32, tag="pm")
                    for h in range(2):
                        nc.tensor.matmul(
                            pm[:, ts(h, 512)], hm[:],
                            xt[:, qq * 1024 + h * 512:qq * 1024 + (h + 1) * 512],
                        )
                    if qtog % 2 == 0:
                        nc.scalar.mul(ot[:, ts(qq, 1024)], pm[:], EVAC_SCALE)
                    else:
                        nc.vector.tensor_scalar_mul(
                            ot[:, ts(qq, 1024)], pm[:], EVAC_SCALE)
                    qtog += 1
                nc.sync.dma_start(o_d[:, c0:c0 + cc], ot[:])

    nc.compile()
    return nc


_NC_CACHE = None


def _get_nc():
    global _NC_CACHE
    if _NC_CACHE is None:
        _NC_CACHE = _build_module()
    return _NC_CACHE


def _in_maps(x: np.ndarray) -> list:
    """Shard, fp8-encode and block-transpose the input for the 8 cores."""
    xs = np.clip(
        np.asarray(x, dtype=np.float32) * np.float32(IN_SCALE),
        -15.5, 15.5,
    )
    xb = xs.astype(FP8)
    hmat = _hadamard_raw().astype(FP8)  # +/- 1, exact
    maps = []
    for c in range(N_CORES):
        shard = xb[c * R_CORE:(c + 1) * R_CORE]          # [1024, 4096]
        dev = shard.reshape(NT, 128, NG, G)              # [t, r, g, c]
        dev = dev.transpose(3, 0, 2, 1).reshape(G, NGC)  # [c, (t, g, r)]
        maps.append({"x": np.ascontiguousarray(dev), "hmat": hmat})
    return maps


def _decode_out(o_dev: np.ndarray) -> np.ndarray:
    """Inverse of the block-transposed layout: [j, (t, g, r)] -> natural."""
    o = o_dev.reshape(G, NT, NG, 128)        # [j, t, g, r]
    return np.ascontiguousarray(
        o.transpose(1, 3, 2, 0).reshape(R_CORE, COLS)
    )


def kernel(x) -> np.ndarray:
    assert x.shape == (ROWS, COLS)
    nc = _get_nc()
    res = run_bass_kernel_spmd(nc, _in_maps(x), core_ids=list(range(N_CORES)))
    out = np.concatenate(
        [_decode_out(r["out"].astype(np.float32)) for r in res.results], axis=0
    )
    # stored value is 2*y; 0.5 is a power of two so this is exact in fp32
    return out * np.float32(HOST_DECODE)
